# revision 1
# baseline (speedup 1.0000x reference)
"""Trainium2 Bass kernel for nn_InvLocalPatOrientConvolution.

Computation:
  1. Host: synthesize the 160-channel 5x5x5 conv filter from
     weight/zeroweight/basis_functions/wigner indices (3.2 MFLOP) and lay out
     per-core operands (fp16).
  2. Device (8 NeuronCores, SPMD): VALID 3D conv as PE matmuls (contraction =
     5 y-taps x 16 input channels = 80 partitions; x/z tap shifts expressed as
     AP offsets into a 5x-replicated SBUF-resident input) + SO(3) grid pooling
     (relu-weighted second-moment ratio) also on the PE.
     Channel split: 128-channel A-tile (full array) + 32-channel B-tile packed
     4 chunks at a time into the 4 PE column groups via tile_position.
     Sharding: batch (2) x output-X-slabs (4) -> 8 cores.
  3. Host: gather per-core slabs into the full (2,16,36,36,36) output.
"""

import os
import sys

for _p in ("/root/.axon_site/_ro/trn_rl_repo", "/opt/trn_rl_repo"):
    if os.path.isdir(_p) and _p not in sys.path:
        sys.path.insert(0, _p)

import numpy as np

import concourse.mybir as mybir
from concourse import bacc
from concourse.tile import TileContext
from concourse.bass_utils import run_bass_kernel_spmd

# Problem constants (hardcoded per harness contract)
ORDER = 2
KS = 5            # conv kernel size
CIN = 16
COUT = 16
EPS = 1e-16
S = 10            # wigner rows
B = 2
D_IN = 40         # input spatial
D_OUT = 36        # output spatial
SLAB = 9          # output X planes per core (36/4)
SLAB_IN = SLAB + KS - 1   # 13 input X planes per core
NCORES = 8
YB = 12           # y-block per chunk
NCHUNK = YB * D_OUT       # 432 columns per matmul chunk

F16 = mybir.dt.float16
F32 = mybir.dt.float32

_prog_cache = {}


def _build_program(repeat=1):
    """Build the SPMD device program (identical on all 8 cores)."""
    nc = bacc.Bacc("TRN2")

    r_d = nc.dram_tensor("r", [SLAB_IN, 80, D_OUT, D_IN], F16, kind="ExternalInput")
    w_d = nc.dram_tensor("w", [25, 80, 160], F16, kind="ExternalInput")
    ga_d = nc.dram_tensor("ga", [128, 4, 108], F16, kind="ExternalInput")
    gb_d = nc.dram_tensor("gb", [128, 108], F16, kind="ExternalInput")
    wnd_d = nc.dram_tensor("wnd", [108, 64], F16, kind="ExternalInput")
    bias_d = nc.dram_tensor("bias", [16, 1], F32, kind="ExternalInput")
    y_d = nc.dram_tensor("y", [16, SLAB, D_OUT, D_OUT], F32, kind="ExternalOutput")

    chunks = [(xr, cy) for xr in range(SLAB) for cy in range(3)]
    groups = [chunks[i:i + 4] for i in range(0, len(chunks), 4)]

    with TileContext(nc) as tc:
        with tc.tile_pool(name="const", bufs=1) as cpool, \
             tc.tile_pool(name="work", bufs=4) as wpool, \
             tc.tile_pool(name="casb", bufs=9) as capool, \
             tc.tile_pool(name="rrel", bufs=10) as rpool, \
             tc.tile_pool(name="conv_ps", bufs=3, space="PSUM") as conv_pool, \
             tc.tile_pool(name="convb_ps", bufs=1, space="PSUM") as convb_pool, \
             tc.tile_pool(name="a_ps", bufs=2, space="PSUM") as a_pool, \
             tc.tile_pool(name="nd_ps", bufs=2, space="PSUM") as nd_pool:

            # ---- resident constants. Order: first 5 input planes + the
            # conv weights (what chunk 0 needs), then the rest — cuts the
            # PE startup stall. Each plane DMA is a contiguous 230KB read.
            rts = []
            for p in range(SLAB_IN):
                rt = cpool.tile([80, D_OUT, D_IN], F16, tag=f"rt{p}")
                rts.append(rt)
            first_dma = [nc.sync, nc.scalar, nc.gpsimd, nc.sync, nc.scalar]
            for p in range(KS):
                first_dma[p].dma_start(out=rts[p][:], in_=r_d[p, :, :, :])
            wt = cpool.tile([80, 25, 160], F16, tag="wt2")
            for ik in range(25):
                nc.sync.dma_start(out=wt[:, ik, :], in_=w_d[ik, :, :])
            gat = cpool.tile([128, 4, 108], F16)
            gbt = cpool.tile([128, 108], F16)
            wndt = cpool.tile([108, 64], F16)
            biast = cpool.tile([16, 1], F32)
            nc.sync.dma_start(out=gat[:], in_=ga_d[:])
            nc.sync.dma_start(out=gbt[:], in_=gb_d[:])
            nc.sync.dma_start(out=wndt[:], in_=wnd_d[:])
            nc.sync.dma_start(out=biast[:], in_=bias_d[:])
            for p in range(KS, SLAB_IN):
                nc.sync.dma_start(out=rts[p][:], in_=r_d[p, :, :, :])

            for _rep in range(repeat):
              pending = None
              for grp in groups:
                # ---- conv A-tiles (128 channels, full array) ----
                ca_sbs = []
                for (xr, cy) in grp:
                    y0 = cy * YB
                    cps = conv_pool.tile([128, NCHUNK], F32, tag="cps")
                    t = 0
                    for i in range(KS):
                        for k in range(KS):
                            rhs = rts[xr + i][:, y0:y0 + YB, k:k + D_OUT]
                            lhsT = wt[:, i * KS + k, 0:128]
                            nc.tensor.matmul(cps[:], lhsT, rhs,
                                             start=(t == 0), stop=(t == 24))
                            t += 1
                    ca = capool.tile([128, NCHUNK], F16, tag="ca")
                    nc.scalar.copy(ca[:], cps[:])
                    ca_sbs.append(ca)

                # previous group's last moment pack: flush here, after ~100N
                # of A-conv (inputs long ready) and right before the equally
                # col-tiled B-conv (no full-width barrier in between)
                if pending is not None:
                    _emit_moments(nc, wndt, biast, wpool, y_d, pending)
                    pending = None

                # ---- conv B-tile (32 channels) col-tiled over the group ----
                cbps = convb_pool.tile([128, NCHUNK], F32, tag="cbps")
                for t, (i, k) in enumerate((i, k) for i in range(KS)
                                           for k in range(KS)):
                    lhsT = wt[:, i * KS + k, 128:160]
                    for c, (xr, cy) in enumerate(grp):
                        y0 = cy * YB
                        rhs = rts[xr + i][:, y0:y0 + YB, k:k + D_OUT]
                        nc.tensor.matmul(
                            cbps[32 * c:32 * (c + 1), :], lhsT, rhs,
                            start=(t == 0), stop=(t == 24),
                            tile_position=(0, 32 * c),
                        )
                cb = capool.tile([128, NCHUNK], F16, tag="cb")
                nc.scalar.copy(cb[:], cbps[:])

                # ---- so3 pooling per chunk (moment stage pipelined one
                # chunk behind the grid stage to hide relu/square latency) ----
                for c, (xr, cy) in enumerate(grp):
                    y0 = cy * YB
                    # num/den partial sums packed into the 4 PE column groups
                    # of ONE psum tile: rows 0-15 / 32-47 = num (mt even/odd),
                    # rows 64-79 / 96-111 = den (mt even/odd).
                    nd_ps = nd_pool.tile([128, NCHUNK], F32, tag="nd")
                    rrels, r2s = [], []
                    for mt in range(4):
                        aps = a_pool.tile([108, NCHUNK], F32, tag="aps")
                        last = (mt == 3)
                        nc.tensor.matmul(aps[:], gat[:, mt, :], ca_sbs[c][:],
                                         start=True, stop=not last)
                        if last:
                            nc.tensor.matmul(
                                aps[:],
                                gbt[32 * c:32 * (c + 1), :],
                                cb[32 * c:32 * (c + 1), :],
                                start=False, stop=True,
                                tile_position=(32 * c, 0),
                            )
                        rrel = rpool.tile([108, NCHUNK], F16, tag="rrel")
                        nc.scalar.activation(rrel[:], aps[:],
                                             mybir.ActivationFunctionType.Relu)
                        r2 = rpool.tile([108, NCHUNK], F16, tag="r2")
                        nc.vector.tensor_mul(r2[:], rrel[:], rrel[:])
                        rrels.append(rrel)
                        r2s.append(r2)
                    cur = (nd_ps, rrels, r2s, xr, y0)
                    if pending is not None:
                        _emit_moments(nc, wndt, biast, wpool, y_d, pending)
                    pending = cur
              if pending is not None:
                  _emit_moments(nc, wndt, biast, wpool, y_d, pending)


    nc.finalize()
    return nc


def _synthesize_filter(weight, zeroweight, basis_functions, wig_w, wig_b):
    """Replicate the reference's kernel synthesis in fp32 numpy.

    Returns kern6[l, e, d, i, j, k] of shape (10, 16, 16, 5, 5, 5)."""
    zero_ext = np.concatenate(
        [zeroweight[None, None],
         np.zeros((ORDER ** 2 - 1, 1, CIN, COUT), weight.dtype)], axis=0)
    wfull = np.concatenate([zero_ext, weight], axis=1)       # (4, 10, 16, 16)
    wg = wfull[wig_w]                                        # (10, 10, 16, 16)
    bg = basis_functions[wig_b]                              # (10, 10, 5, 5, 5)
    kern6 = np.einsum("lred,lrijk->ledijk", wg, bg)          # (10,16,16,5,5,5)
    return np.ascontiguousarray(kern6.astype(np.float32))


def _host_prep(x, weight, zeroweight, bias, so3basisgrid, w_i,
               basis_functions, wig_w, wig_b):
    kern6 = _synthesize_filter(weight, zeroweight, basis_functions, wig_w, wig_b)

    # conv weights: W[i*5+k, j*16+d, e*10+l]  (cols e-major; A = cols 0..127)
    w_arr = np.ascontiguousarray(
        kern6.transpose(3, 5, 4, 2, 1, 0).reshape(25, 80, 160)).astype(np.float16)

    g2 = so3basisgrid.reshape(27, S).astype(np.float32)      # [mln, l]
    g2t16 = g2.T.astype(np.float16)                          # [l, mln]

    # A-tile so3 lhsT: ga[p, mt, el2*27+mln]; p = e*10+l (only p < 128 rows
    # live in the A conv tile). mt covers e in [4mt, 4mt+4).
    ga = np.zeros((128, 4, 108), np.float16)
    for mt in range(4):
        for el2 in range(4):
            e = 4 * mt + el2
            for l in range(S):
                p = e * S + l
                if p < 128:
                    ga[p, mt, el2 * 27:(el2 + 1) * 27] = g2t16[l]
    # B-tile so3 lhsT (only mt=3, e 12..15), replicated per chunk slot:
    # B row r: r=0,1 -> (e12, l8+r); r=2+10*m+l -> (e13+m, l)
    gb = np.zeros((128, 108), np.float16)
    for cslot in range(4):
        for r in range(32):
            if r < 2:
                e, l = 12, 8 + r
            else:
                e, l = 13 + (r - 2) // S, (r - 2) % S
            el2 = e - 12
            gb[32 * cslot + r, el2 * 27:(el2 + 1) * 27] = g2t16[l]

    # weighted-moment lhsT: wnd[(el2*27+mln), mt*16+e], e = 4mt+el2
    w_flat = np.asarray(w_i, np.float32)[(np.arange(27) // 3) % 3]
    wnd = np.zeros((108, 4, 16), np.float16)
    for mt in range(4):
        for el2 in range(4):
            e = 4 * mt + el2
            wnd[el2 * 27:(el2 + 1) * 27, mt, e] = w_flat.astype(np.float16)
    wnd = wnd.reshape(108, 64)

    bias_arr = np.asarray(bias, np.float32).reshape(16, 1)

    in_maps = []
    for c in range(NCORES):
        b, q = divmod(c, 4)
        slab = x[b, :, q * SLAB:q * SLAB + SLAB_IN]          # (16, 13, 40, 40)
        r_arr = np.empty((SLAB_IN, 5, 16, D_OUT, D_IN), np.float16)
        for j in range(5):
            r_arr[:, j] = slab[:, :, j:j + D_OUT, :].transpose(1, 0, 2, 3)
        in_maps.append({
            "r": np.ascontiguousarray(r_arr.reshape(SLAB_IN, 80, D_OUT, D_IN)),
            "w": w_arr,
            "ga": np.ascontiguousarray(ga),
            "gb": np.ascontiguousarray(gb),
            "wnd": np.ascontiguousarray(wnd),
            "bias": bias_arr,
        })
    return in_maps


def _run(inputs, trace=False, **run_kwargs):
    inputs = {k: np.asarray(v) for k, v in inputs.items()}
    in_maps = _host_prep(**inputs)
    if "nc" not in _prog_cache:
        _prog_cache["nc"] = _build_program()
    nc = _prog_cache["nc"]
    try:
        res = run_bass_kernel_spmd(nc, in_maps, core_ids=list(range(NCORES)),
                                   trace=trace, **run_kwargs)
    except ModuleNotFoundError as e:
        if "axon_hooks" not in str(e):
            raise
        # Tracing requested (e.g. BASS_TRACE=1) but this axon client has no
        # NTFF profile hook — rerun with tracing disabled.
        os.environ["BASS_NEVER_TRACE"] = "1"
        res = run_bass_kernel_spmd(nc, in_maps, core_ids=list(range(NCORES)),
                                   trace=False, **run_kwargs)
    out = np.empty((B, COUT, D_OUT, D_OUT, D_OUT), np.float32)
    for c in range(NCORES):
        b, q = divmod(c, 4)
        out[b, :, q * SLAB:(q + 1) * SLAB] = res.results[c]["y"]
    return out, res


def kernel(**inputs):
    out, _ = _run(inputs)
    return out


def _emit_moments(nc, wndt, biast, wpool, y_d, st):
    """Emit the 8 col-group-packed moment matmuls + finalize + store for a
    chunk whose grid stage (a/relu/square) was already emitted."""
    nd_ps, rrels, r2s, xr, y0 = st
    for mt in range(4):
        wnd_g = wndt[:, mt * 16:(mt + 1) * 16]
        cg = 32 * (mt % 2)
        nc.tensor.matmul(nd_ps[cg:cg + 16, :], wnd_g, r2s[mt][:],
                         start=(mt < 2), stop=(mt >= 2),
                         tile_position=(0, cg))
        nc.tensor.matmul(nd_ps[64 + cg:64 + cg + 16, :], wnd_g, rrels[mt][:],
                         start=(mt < 2), stop=(mt >= 2),
                         tile_position=(0, 64 + cg))

    num_a = wpool.tile([16, NCHUNK], F32, tag="num_a")
    nc.scalar.copy(num_a[:], nd_ps[0:16, :])
    den_a = wpool.tile([16, NCHUNK], F32, tag="den_a")
    nc.scalar.activation(den_a[:], nd_ps[64:80, :],
         mybir.ActivationFunctionType.Copy,
         bias=EPS)
    num_sb = wpool.tile([16, NCHUNK], F32, tag="num_sb")
    nc.vector.tensor_add(num_sb[:], num_a[:], nd_ps[32:48, :])
    den_sb = wpool.tile([16, NCHUNK], F32, tag="den_sb")
    nc.vector.tensor_add(den_sb[:], den_a[:], nd_ps[96:112, :])
    recip = wpool.tile([16, NCHUNK], F32, tag="recip")
    nc.vector.reciprocal(recip[:], den_sb[:])
    out_sb = wpool.tile([16, NCHUNK], F32, tag="out_sb")
    nc.vector.tensor_mul(out_sb[:], num_sb[:], recip[:])
    nc.vector.tensor_scalar_add(out_sb[:], out_sb[:],
                biast[:, 0:1])

    dst = y_d[:, xr, y0:y0 + YB, :]
    nc.sync.dma_start(out=dst, in_=out_sb[:].rearrange(
        "p (a b) -> p a b", a=YB))



# revision 4
# speedup vs baseline: 1.4249x; 1.4249x over previous
"""Trainium2 Bass kernel for nn_InvLocalPatOrientConvolution.

Computation:
  1. Host: synthesize the 160-channel 5x5x5 conv filter from
     weight/zeroweight/basis_functions/wigner indices, fold the so3 grid into
     the e>=12 channels (direct 108-channel "B" conv), and lay out per-core
     operands (fp16).
  2. Device (8 NeuronCores, SPMD):
     - conv A (120 ch = e<12 x l) + conv B-direct (108 ch = (el2,mln) for
       e>=12, grid pre-applied) as PE matmuls with K=128 packing: input rows
       are (i,j,d) combos (25 tap-pairs x 16 ch = 400 rows) split into three
       128-row groups + an 80-row z-preshifted leftover tile, so each chunk
       needs 16 matmuls per output tile instead of 25.
     - grid stage mt0..2 (324 outs) from the A channels, 3 matmuls.
     - relu (scalar) + square (vector) into (108,4,432) mega-tiles; moment
       stage = 2x4 chained matmuls (num/den) packed into one PSUM bank.
     - finalize num/(den+eps)+bias on DVE, one chunk behind the grid stage.
     Sharding: batch (2) x output-X-slabs (4) -> 8 cores.
  3. Host: gather per-core slabs into the full (2,16,36,36,36) output.
"""

import os
import sys

for _p in ("/root/.axon_site/_ro/trn_rl_repo", "/opt/trn_rl_repo"):
    if os.path.isdir(_p) and _p not in sys.path:
        sys.path.insert(0, _p)

import numpy as np

import concourse.mybir as mybir
from concourse import bacc
from concourse.tile import TileContext
from concourse.bass_utils import run_bass_kernel_spmd

# Problem constants (hardcoded per harness contract)
ORDER = 2
KS = 5            # conv kernel size
CIN = 16
COUT = 16
EPS = 1e-16
S = 10            # wigner rows
B = 2
D_IN = 40         # input spatial
D_OUT = 36        # output spatial
SLAB = 9          # output X planes per core (36/4)
SLAB_IN = SLAB + KS - 1   # 13 input X planes per core
NCORES = 8
YB = 12           # y-block per chunk
NCH = YB * D_OUT  # 432 columns per matmul chunk

F16 = mybir.dt.float16
F32 = mybir.dt.float32

_prog_cache = {}


def _build_program():
    """Build the SPMD device program (identical on all 8 cores)."""
    nc = bacc.Bacc("TRN2")

    r_ds = [nc.dram_tensor(f"r{g}", [128, SLAB, D_OUT, D_IN], F16,
                           kind="ExternalInput") for g in range(3)]
    rl_d = nc.dram_tensor("rl", [80, SLAB, D_OUT, D_OUT], F16,
                          kind="ExternalInput")
    wa_d = nc.dram_tensor("wa", [128, KS, 3, 120], F16, kind="ExternalInput")
    wb_d = nc.dram_tensor("wb", [128, KS, 3, 108], F16, kind="ExternalInput")
    wla_d = nc.dram_tensor("wla", [80, 120], F16, kind="ExternalInput")
    wlb_d = nc.dram_tensor("wlb", [80, 108], F16, kind="ExternalInput")
    gat_d = nc.dram_tensor("gat", [120, 3, 108], F16, kind="ExternalInput")
    wnd_d = nc.dram_tensor("wnd", [108, 4, 16], F16, kind="ExternalInput")
    bias_d = nc.dram_tensor("bias", [16, 1], F32, kind="ExternalInput")
    y_d = nc.dram_tensor("y", [16, SLAB, D_OUT, D_OUT], F32,
                         kind="ExternalOutput")

    chunks = [(xr, cy) for xr in range(SLAB) for cy in range(3)]

    with TileContext(nc) as tc:
        with tc.tile_pool(name="const", bufs=1) as cpool, \
             tc.tile_pool(name="ca", bufs=2) as capool, \
             tc.tile_pool(name="rr", bufs=4) as rpool, \
             tc.tile_pool(name="fin", bufs=3) as wpool, \
             tc.tile_pool(name="a_ps", bufs=1, space="PSUM") as a_pool, \
             tc.tile_pool(name="b_ps", bufs=1, space="PSUM") as b_pool, \
             tc.tile_pool(name="g_ps", bufs=1, space="PSUM") as g_pool, \
             tc.tile_pool(name="nd_ps", bufs=2, space="PSUM") as nd_pool:

            # ---- resident constants.
            wat = cpool.tile([128, KS, 3, 120], F16, tag="wat")
            wbt = cpool.tile([128, KS, 3, 108], F16, tag="wbt")
            wlat = cpool.tile([80, 120], F16, tag="wlat")
            wlbt = cpool.tile([80, 108], F16, tag="wlbt")
            gatt = cpool.tile([120, 3, 108], F16, tag="gatt")
            wndt = cpool.tile([108, 4, 16], F16, tag="wndt")
            biast = cpool.tile([16, 1], F32, tag="biast")
            nc.sync.dma_start(out=wat[:], in_=wa_d[:])
            nc.sync.dma_start(out=wbt[:], in_=wb_d[:])
            nc.sync.dma_start(out=wlat[:], in_=wla_d[:])
            nc.sync.dma_start(out=wlbt[:], in_=wlb_d[:])
            nc.sync.dma_start(out=gatt[:], in_=gat_d[:])
            nc.sync.dma_start(out=wndt[:], in_=wnd_d[:])
            nc.sync.dma_start(out=biast[:], in_=bias_d[:])

            rts = [cpool.tile([128, SLAB, D_OUT, D_IN], F16, tag=f"rt{g}",
                              name=f"rt{g}")
                   for g in range(3)]
            rlt = cpool.tile([80, SLAB, D_OUT, D_OUT], F16, tag="rlt")
            # per-xr slices so early chunks only wait on their own planes
            for xr in range(SLAB):
                for g in range(3):
                    nc.sync.dma_start(out=rts[g][:, xr], in_=r_ds[g][:, xr])
                nc.sync.dma_start(out=rlt[:, xr], in_=rl_d[:, xr])

            pending = None
            for (xr, cy) in chunks:
                y0 = cy * YB

                # ---- conv A (120 ch) and conv B-direct (108 ch) ----
                aps = a_pool.tile([120, NCH], F32, tag="aps")
                bps = b_pool.tile([108, NCH], F32, tag="bps")
                for out_ps, wt, wlt in ((aps, wat, wlat), (bps, wbt, wlbt)):
                    t = 0
                    for k in range(KS):
                        for g in range(3):
                            rhs = rts[g][:, xr, y0:y0 + YB, k:k + D_OUT]
                            nc.tensor.matmul(out_ps[:], wt[:, k, g, :], rhs,
                                             start=(t == 0), stop=False)
                            t += 1
                    nc.tensor.matmul(out_ps[:], wlt[:],
                                     rlt[:, xr, y0:y0 + YB, :],
                                     start=False, stop=True)

                # conv-A psum -> SBUF fp16 for the grid stage
                ca = capool.tile([120, NCH], F16, tag="ca")
                nc.scalar.copy(ca[:], aps[:])

                # previous chunk's moment/finalize work: emitted here so the
                # PE never waits on the relu/square of the current chunk
                if pending is not None:
                    _emit_moments(nc, wndt, biast, wpool, nd_pool, y_d,
                                  pending)

                # ---- grid stage mt0..2 (3 matmuls, one PSUM bank each) ----
                gps = g_pool.tile([108, 3, 512], F32, tag="gps")
                for mt in range(3):
                    nc.tensor.matmul(gps[:, mt, 0:NCH], gatt[:, mt, :], ca[:],
                                     start=True, stop=True)

                # ---- relu (scalar) + square (vector) ----
                rrel = rpool.tile([108, 4, NCH], F16, tag="rrel")
                nc.scalar.activation(rrel[:, 0:3, :], gps[:, :, 0:NCH],
                                     mybir.ActivationFunctionType.Relu)
                nc.scalar.activation(rrel[:, 3, :], bps[:],
                                     mybir.ActivationFunctionType.Relu)
                r2 = rpool.tile([108, 4, NCH], F16, tag="r2")
                nc.vector.tensor_mul(r2[:], rrel[:], rrel[:])
                pending = (rrel, r2, xr, y0)

            if pending is not None:
                _emit_moments(nc, wndt, biast, wpool, nd_pool, y_d, pending)

    nc.finalize()
    return nc


def _emit_moments(nc, wndt, biast, wpool, nd_pool, y_d, st):
    """Moment matmuls (num from r2, den from rrel; 4 chained each, packed in
    one PSUM bank at column positions 0/64) + finalize + store."""
    rrel, r2, xr, y0 = st
    nd = nd_pool.tile([128, NCH], F32, tag="nd")
    for mt in range(4):
        nc.tensor.matmul(nd[0:16, :], wndt[:, mt, :], r2[:, mt, :],
                         start=(mt == 0), stop=(mt == 3),
                         tile_position=(0, 0))
        nc.tensor.matmul(nd[64:80, :], wndt[:, mt, :], rrel[:, mt, :],
                         start=(mt == 0), stop=(mt == 3),
                         tile_position=(0, 64))
    den_sb = wpool.tile([16, NCH], F32, tag="den_sb")
    nc.scalar.activation(den_sb[:], nd[64:80, :],
                         mybir.ActivationFunctionType.Copy, bias=EPS)
    recip = wpool.tile([16, NCH], F32, tag="recip")
    nc.vector.reciprocal(recip[:], den_sb[:])
    out_sb = wpool.tile([16, NCH], F32, tag="out_sb")
    nc.vector.tensor_mul(out_sb[:], nd[0:16, :], recip[:])
    nc.vector.tensor_scalar_add(out_sb[:], out_sb[:], biast[:, 0:1])
    nc.sync.dma_start(out=y_d[:, xr, y0:y0 + YB, :],
                      in_=out_sb[:].rearrange("p (a b) -> p a b", a=YB))


def _synthesize_filter(weight, zeroweight, basis_functions, wig_w, wig_b):
    """Replicate the reference's kernel synthesis in fp32 numpy.

    Returns kern6[l, e, d, i, j, k] of shape (10, 16, 16, 5, 5, 5) where
    (e,l) indexes the 160 conv output channels and (d,i,j,k) the
    contraction."""
    zero_ext = np.concatenate(
        [zeroweight[None, None],
         np.zeros((ORDER ** 2 - 1, 1, CIN, COUT), weight.dtype)], axis=0)
    wfull = np.concatenate([zero_ext, weight], axis=1)       # (4, 10, 16, 16)
    wg = wfull[wig_w]                                        # (10, 10, 16, 16)
    bg = basis_functions[wig_b]                              # (10, 10, 5, 5, 5)
    kern6 = np.einsum("lred,lrijk->ledijk", wg, bg)          # (10,16,16,5,5,5)
    return np.ascontiguousarray(kern6.astype(np.float32))


def _host_prep(x, weight, zeroweight, bias, so3basisgrid, w_i,
               basis_functions, wig_w, wig_b):
    kern6 = _synthesize_filter(weight, zeroweight, basis_functions,
                               wig_w, wig_b)
    # Wf[pair(i,j), d, k, out(e*10+l)]
    Wf = np.ascontiguousarray(
        kern6.transpose(3, 4, 2, 5, 1, 0).reshape(25, 16, KS, 160))

    g2 = np.asarray(so3basisgrid, np.float32).reshape(27, S)  # [mln, l]
    w_flat = np.asarray(w_i, np.float32)[(np.arange(27) // 3) % 3]

    # B channels (e>=12): fold grid -> out (el2*27+mln)
    WfB = Wf[:, :, :, 120:].reshape(25, 16, KS, 4, S)        # (..., el2, l)
    kern2B = np.einsum("pdkel,ml->pdkem", WfB, g2).reshape(25, 16, KS, 108)

    # conv lhsT tiles: row rho = pair*16+d -> group g = rho//128, p = rho%128
    wa = np.zeros((128, KS, 3, 120), np.float16)
    wb = np.zeros((128, KS, 3, 108), np.float16)
    for pair in range(24):
        g, p0 = divmod(pair * 16, 128)
        wa[p0:p0 + 16, :, g, :] = Wf[pair, :, :, 0:120]
        wb[p0:p0 + 16, :, g, :] = kern2B[pair]
    wla = np.zeros((80, 120), np.float16)
    wlb = np.zeros((80, 108), np.float16)
    for k in range(KS):
        wla[16 * k:16 * k + 16, :] = Wf[24, :, k, 0:120]
        wlb[16 * k:16 * k + 16, :] = kern2B[24, :, k, :]

    # grid lhsT for mt0..2: rows (e*10+l, e<12)
    gat = np.zeros((120, 3, 108), np.float16)
    for mt in range(3):
        for el2 in range(4):
            e = 4 * mt + el2
            for l in range(S):
                gat[e * S + l, mt, el2 * 27:(el2 + 1) * 27] = g2[:, l]

    # moment lhsT: rows (el2*27+mln) -> col e = 4*mt+el2
    wnd = np.zeros((108, 4, 16), np.float16)
    for mt in range(4):
        for el2 in range(4):
            e = 4 * mt + el2
            wnd[el2 * 27:(el2 + 1) * 27, mt, e] = w_flat

    bias_arr = np.asarray(bias, np.float32).reshape(16, 1)

    in_maps = []
    for c in range(NCORES):
        b, q = divmod(c, 4)
        slab = np.asarray(x[b, :, q * SLAB:q * SLAB + SLAB_IN], np.float16)
        rs = [np.zeros((128, SLAB, D_OUT, D_IN), np.float16)
              for _ in range(3)]
        for pair in range(24):
            i, j = divmod(pair, KS)
            g, p0 = divmod(pair * 16, 128)
            rs[g][p0:p0 + 16] = slab[:, i:i + SLAB, j:j + D_OUT, :]
        rl = np.empty((80, SLAB, D_OUT, D_OUT), np.float16)
        i24, j24 = 4, 4
        for k in range(KS):
            rl[16 * k:16 * k + 16] = slab[:, i24:i24 + SLAB,
                                          j24:j24 + D_OUT, k:k + D_OUT]
        in_maps.append({
            "r0": rs[0], "r1": rs[1], "r2": rs[2], "rl": rl,
            "wa": wa, "wb": wb, "wla": wla, "wlb": wlb,
            "gat": np.ascontiguousarray(gat),
            "wnd": np.ascontiguousarray(wnd),
            "bias": bias_arr,
        })
    return in_maps


def _run(inputs, trace=False, **run_kwargs):
    inputs = {k: np.asarray(v) for k, v in inputs.items()}
    in_maps = _host_prep(**inputs)
    if "nc" not in _prog_cache:
        _prog_cache["nc"] = _build_program()
    nc = _prog_cache["nc"]
    try:
        res = run_bass_kernel_spmd(nc, in_maps, core_ids=list(range(NCORES)),
                                   trace=trace, **run_kwargs)
    except ModuleNotFoundError as e:
        if "axon_hooks" not in str(e):
            raise
        os.environ["BASS_NEVER_TRACE"] = "1"
        res = run_bass_kernel_spmd(nc, in_maps, core_ids=list(range(NCORES)),
                                   trace=False, **run_kwargs)
    out = np.empty((B, COUT, D_OUT, D_OUT, D_OUT), np.float32)
    for c in range(NCORES):
        b, q = divmod(c, 4)
        out[b, :, q * SLAB:(q + 1) * SLAB] = res.results[c]["y"]
    return out, res


def kernel(**inputs):
    out, _ = _run(inputs)
    return out


# revision 15
# speedup vs baseline: 1.6232x; 1.1392x over previous
"""Trainium2 Bass kernel for nn_InvLocalPatOrientConvolution.

Computation:
  1. Host: synthesize the 160-channel 5x5x5 conv filter from
     weight/zeroweight/basis_functions/wigner indices, fold the so3 grid into
     the e>=12 channels (direct 108-channel "B" conv), quantize weights and
     input to fp8-e4m3 hi/lo pairs, and lay out per-core operands.
  2. Device (8 NeuronCores, SPMD):
     - conv A (120 ch = e<12 x l) + conv B-direct (108 ch = (el2,mln) for
       e>=12, grid pre-applied) as fp8 DoubleRow PE matmuls. Input rows are
       (i,j,d) combos (25 tap-pairs x 16 ch = 400 rows). Per output tile:
       term1 = [w_hi|w_hi] . [x_hi|x_lo]  (exact in x; 15 (k,group) matmuls
       + 1 z-preshifted leftover-pair matmul), term2 = w_lo . x_hi with
       256-row k-tile packing (10 matmuls). The dropped w_lo.x_lo term is
       O(eps^2). Global power-of-2 weight scales are undone in the fp16
       grid / moment lhsTs.
     - grid stage mt0..2 (324 outs) from the A channels, 3 fp16 matmuls.
     - relu (scalar) + square (vector) into (108,4,432) fp16 mega-tiles;
       moment stage = 2x4 chained fp16 matmuls (num/den) in one PSUM bank.
     - finalize num/(den+eps)+bias on DVE, one chunk behind the grid stage.
     Sharding: batch (2) x output-X-slabs (4) -> 8 cores.
  3. Host: gather per-core slabs into the full (2,16,36,36,36) output.
"""

import os
import sys

for _p in ("/root/.axon_site/_ro/trn_rl_repo", "/opt/trn_rl_repo"):
    if os.path.isdir(_p) and _p not in sys.path:
        sys.path.insert(0, _p)

import numpy as np
import ml_dtypes

import concourse.mybir as mybir
from concourse import bacc
from concourse.tile import TileContext
from concourse.bass_utils import run_bass_kernel_spmd

# Problem constants (hardcoded per harness contract)
ORDER = 2
KS = 5            # conv kernel size
CIN = 16
COUT = 16
EPS = 1e-16
S = 10            # wigner rows
B = 2
D_IN = 40         # input spatial
D_OUT = 36        # output spatial
SLAB = 9          # output X planes per core (36/4)
SLAB_IN = SLAB + KS - 1   # 13 input X planes per core
NCORES = 8
YB = 12           # y-block per chunk
NCH = YB * D_OUT  # 432 columns per matmul chunk

F16 = mybir.dt.float16
F32 = mybir.dt.float32
F8 = mybir.dt.float8e4
E4M3 = ml_dtypes.float8_e4m3
DR = mybir.MatmulPerfMode.DoubleRow

_prog_cache = {}


def _build_program(inv_sB=1.0):
    """Build the SPMD device program (identical on all 8 cores)."""
    nc = bacc.Bacc("TRN2")

    rhl_ds = [nc.dram_tensor(f"rhl{g}", [128, SLAB, 2, D_IN, D_OUT], F8,
                             kind="ExternalInput") for g in range(3)]
    rl_d = nc.dram_tensor("rl", [80, SLAB, 2, D_OUT, D_OUT], F8,
                          kind="ExternalInput")
    rh2_ds = [nc.dram_tensor(f"rh2{t}", [128, SLAB, 2, D_IN, D_OUT], F8,
                             kind="ExternalInput") for t in range(2)]
    wahi_d = nc.dram_tensor("wahi", [128, KS, 3, 2, 128], F8,
                            kind="ExternalInput")
    wbhi_d = nc.dram_tensor("wbhi", [128, KS, 3, 2, 112], F8,
                            kind="ExternalInput")
    wlahi_d = nc.dram_tensor("wlahi", [80, 2, 128], F8, kind="ExternalInput")
    wlbhi_d = nc.dram_tensor("wlbhi", [80, 2, 112], F8, kind="ExternalInput")
    walo_d = nc.dram_tensor("walo", [128, KS, 2, 2, 128], F8,
                            kind="ExternalInput")
    wblo_d = nc.dram_tensor("wblo", [128, KS, 2, 2, 112], F8,
                            kind="ExternalInput")
    gat_d = nc.dram_tensor("gat", [120, 3, 108], F16, kind="ExternalInput")
    wnd_d = nc.dram_tensor("wnd", [108, 4, 16], F16, kind="ExternalInput")
    bias_d = nc.dram_tensor("bias", [16, 1], F32, kind="ExternalInput")
    y_d = nc.dram_tensor("y", [16, SLAB, D_OUT, D_OUT], F32,
                         kind="ExternalOutput")

    chunks = [(xr, cy) for xr in range(SLAB) for cy in range(3)]

    with TileContext(nc) as tc:
        with tc.tile_pool(name="const", bufs=1) as cpool, \
             tc.tile_pool(name="ca", bufs=2) as capool, \
             tc.tile_pool(name="rr", bufs=4) as rpool, \
             tc.tile_pool(name="fin", bufs=2) as wpool, \
             tc.tile_pool(name="a_ps", bufs=1, space="PSUM") as a_pool, \
             tc.tile_pool(name="b_ps", bufs=1, space="PSUM") as b_pool, \
             tc.tile_pool(name="g_ps", bufs=1, space="PSUM") as g_pool, \
             tc.tile_pool(name="nd_ps", bufs=2, space="PSUM") as nd_pool:

            # ---- resident tiles. DMA order: chunk-0 deps first.
            rhls = [cpool.tile([128, SLAB, 2, D_IN, D_OUT], F8, tag=f"rhl{g}",
                               name=f"rhl{g}") for g in range(3)]
            rlt = cpool.tile([80, SLAB, 2, D_OUT, D_OUT], F8, tag="rlt")
            rh2s = [cpool.tile([128, SLAB, 2, D_IN, D_OUT], F8, tag=f"rh2{t}",
                               name=f"rh2{t}") for t in range(2)]
            wahit = cpool.tile([128, KS, 3, 2, 128], F8, tag="wahit")
            wbhit = cpool.tile([128, KS, 3, 2, 112], F8, tag="wbhit")
            wlahit = cpool.tile([80, 2, 128], F8, tag="wlahit")
            wlbhit = cpool.tile([80, 2, 112], F8, tag="wlbhit")
            walot = cpool.tile([128, KS, 2, 2, 128], F8, tag="walot")
            wblot = cpool.tile([128, KS, 2, 2, 112], F8, tag="wblot")
            gatt = cpool.tile([120, 3, 108], F16, tag="gatt")
            wndt = cpool.tile([108, 4, 16], F16, tag="wndt")
            biast = cpool.tile([16, 1], F32, tag="biast")

            # chunk-0 input slices + conv weights first
            for g in range(3):
                nc.sync.dma_start(out=rhls[g][:, 0], in_=rhl_ds[g][:, 0])
            nc.sync.dma_start(out=rlt[:, 0], in_=rl_d[:, 0])
            for t in range(2):
                nc.sync.dma_start(out=rh2s[t][:, 0], in_=rh2_ds[t][:, 0])
            nc.sync.dma_start(out=wahit[:], in_=wahi_d[:])
            nc.sync.dma_start(out=wlahit[:], in_=wlahi_d[:])
            nc.sync.dma_start(out=walot[:], in_=walo_d[:])
            nc.sync.dma_start(out=wbhit[:], in_=wbhi_d[:])
            nc.sync.dma_start(out=wlbhit[:], in_=wlbhi_d[:])
            nc.sync.dma_start(out=wblot[:], in_=wblo_d[:])
            nc.sync.dma_start(out=gatt[:], in_=gat_d[:])
            nc.sync.dma_start(out=wndt[:], in_=wnd_d[:])
            nc.sync.dma_start(out=biast[:], in_=bias_d[:])
            for xr in range(1, SLAB):
                for g in range(3):
                    nc.sync.dma_start(out=rhls[g][:, xr], in_=rhl_ds[g][:, xr])
                nc.sync.dma_start(out=rlt[:, xr], in_=rl_d[:, xr])
                for t in range(2):
                    nc.sync.dma_start(out=rh2s[t][:, xr], in_=rh2_ds[t][:, xr])

            pending = None
            for (xr, cy) in chunks:
                y0 = cy * YB

                # ---- conv A (120 ch) and conv B-direct (108 ch) ----
                aps = a_pool.tile([120, NCH], F32, tag="aps")
                bps = b_pool.tile([108, NCH], F32, tag="bps")
                for out_ps, m, whi, wlhi, wlo in (
                        (aps, 120, wahit, wlahit, walot),
                        (bps, 108, wbhit, wlbhit, wblot)):
                    t = 0
                    for j in range(KS):
                        for g in range(3):
                            rhs = rhls[g][:, xr, 0:2, y0 + j:y0 + j + YB, :]
                            nc.tensor.matmul(out_ps[:],
                                             whi[:, j, g, :, 0:m],
                                             rhs, start=(t == 0), stop=False,
                                             perf_mode=DR)
                            t += 1
                        for t2 in range(2):
                            rhs = rh2s[t2][:, xr, 0:2,
                                           y0 + j:y0 + j + YB, :]
                            nc.tensor.matmul(out_ps[:],
                                             wlo[:, j, t2, :, 0:m],
                                             rhs, start=False, stop=False,
                                             perf_mode=DR)
                    nc.tensor.matmul(out_ps[:], wlhi[:, :, 0:m],
                                     rlt[:, xr, 0:2, y0:y0 + YB, :],
                                     start=False, stop=True, perf_mode=DR)

                # conv-A psum -> SBUF fp16 for the grid stage
                ca = capool.tile([120, NCH], F16, tag="ca")
                nc.scalar.copy(ca[:], aps[:])

                # previous chunk's moment/finalize work: emitted here so the
                # PE never waits on the relu/square of the current chunk
                if pending is not None:
                    _emit_moments(nc, wndt, biast, wpool, nd_pool,
                                  y_d, pending)

                # ---- grid stage mt0..2 (3 matmuls, one PSUM bank each) ----
                gps = g_pool.tile([108, 3, 512], F32, tag="gps")
                for mt in range(3):
                    nc.tensor.matmul(gps[:, mt, 0:NCH], gatt[:, mt, :], ca[:],
                                     start=True, stop=True)

                # ---- relu (scalar) + square (vector) ----
                rrel = rpool.tile([108, 4, NCH], F16, tag="rrel")
                nc.scalar.activation(rrel[:, 0:3, :], gps[:, :, 0:NCH],
                                     mybir.ActivationFunctionType.Relu)
                nc.scalar.activation(rrel[:, 3, :], bps[:],
                                     mybir.ActivationFunctionType.Relu,
                                     scale=inv_sB)
                r2 = rpool.tile([108, 4, NCH], F16, tag="r2")
                nc.vector.tensor_mul(r2[:], rrel[:], rrel[:])
                pending = (rrel, r2, xr, y0)

            if pending is not None:
                _emit_moments(nc, wndt, biast, wpool, nd_pool, y_d,
                              pending)

    nc.finalize()
    return nc


def _emit_moments(nc, wndt, biast, wpool, nd_pool, y_d, st):
    """Moment matmuls (num from r2, den from rrel; 4 chained each, packed in
    one PSUM bank at column positions 0/64) + finalize + store."""
    rrel, r2, xr, y0 = st
    nd = nd_pool.tile([128, NCH], F32, tag="nd")
    for mt in range(4):
        nc.tensor.matmul(nd[0:16, :], wndt[:, mt, :], r2[:, mt, :],
                         start=(mt == 0), stop=(mt == 3),
                         tile_position=(0, 0))
        nc.tensor.matmul(nd[64:80, :], wndt[:, mt, :], rrel[:, mt, :],
                         start=(mt == 0), stop=(mt == 3),
                         tile_position=(0, 64))
    den_sb = wpool.tile([16, NCH], F32, tag="den_sb")
    nc.scalar.activation(den_sb[:], nd[64:80, :],
                         mybir.ActivationFunctionType.Copy, bias=EPS)
    recip = wpool.tile([16, NCH], F32, tag="recip")
    nc.vector.reciprocal(recip[:], den_sb[:])
    out_sb = wpool.tile([16, NCH], F32, tag="out_sb")
    nc.vector.tensor_mul(out_sb[:], nd[0:16, :], recip[:])
    nc.vector.tensor_scalar_add(out_sb[:], out_sb[:], biast[:, 0:1])
    nc.sync.dma_start(out=y_d[:, xr, y0:y0 + YB, :],
                      in_=out_sb[:].rearrange("p (a b) -> p a b", a=YB))


def _synthesize_filter(weight, zeroweight, basis_functions, wig_w, wig_b):
    """Replicate the reference's kernel synthesis in fp32 numpy.

    Returns kern6[l, e, d, i, j, k] of shape (10, 16, 16, 5, 5, 5) where
    (e,l) indexes the 160 conv output channels and (d,i,j,k) the
    contraction."""
    zero_ext = np.concatenate(
        [zeroweight[None, None],
         np.zeros((ORDER ** 2 - 1, 1, CIN, COUT), weight.dtype)], axis=0)
    wfull = np.concatenate([zero_ext, weight], axis=1)       # (4, 10, 16, 16)
    wg = wfull[wig_w]                                        # (10, 10, 16, 16)
    bg = basis_functions[wig_b]                              # (10, 10, 5, 5, 5)
    kern6 = np.einsum("lred,lrijk->ledijk", wg, bg)          # (10,16,16,5,5,5)
    return np.ascontiguousarray(kern6.astype(np.float32))


def _q8_pair(a):
    hi = a.astype(E4M3)
    lo = (a - hi.astype(np.float32)).astype(E4M3)
    return hi, lo


def _pow2_scale(absmax, target=64.0):
    if absmax <= 0:
        return 1.0
    return 2.0 ** np.floor(np.log2(target / absmax))


def _host_prep(x, weight, zeroweight, bias, so3basisgrid, w_i,
               basis_functions, wig_w, wig_b):
    kern6 = _synthesize_filter(weight, zeroweight, basis_functions,
                               wig_w, wig_b)
    # Wf[pair(i,j), d, k, out(e*10+l)]
    Wf = np.ascontiguousarray(
        kern6.transpose(3, 4, 2, 5, 1, 0).reshape(25, 16, KS, 160))

    g2 = np.asarray(so3basisgrid, np.float32).reshape(27, S)  # [mln, l]
    w_flat = np.asarray(w_i, np.float32)[(np.arange(27) // 3) % 3]

    # B channels (e>=12): fold grid -> out (el2*27+mln)
    WfB = Wf[:, :, :, 120:].reshape(25, 16, KS, 4, S)        # (..., el2, l)
    kern2B = np.einsum("pdkel,ml->pdkem", WfB, g2).reshape(25, 16, KS, 108)
    WfA = Wf[:, :, :, 0:120]

    sA = _pow2_scale(np.abs(WfA).max())
    sB = _pow2_scale(np.abs(kern2B).max())
    WfA_hi, WfA_lo = _q8_pair(WfA * sA)          # (25, 16, KS, 120) e4m3
    kB_hi, kB_lo = _q8_pair(kern2B * sB)

    # conv lhsT tiles. Row space: rho = pair2*16+d with pair2 = i*5+k; the
    # j-tap picks the weight slice and the rhs y-offset. M padded to mp for
    # the 16B dual-fp8 pair-stride rule.
    # W2[pair2, d, j, out] = Wf[i*5+j, d, k, out]
    def reindex(w):
        m = w.shape[-1]
        w5 = w.reshape(KS, KS, 16, KS, m)                # (i, j, d, k, m)
        return np.ascontiguousarray(
            w5.transpose(0, 3, 2, 1, 4).reshape(25, 16, KS, m))

    def pack_hi(w_hi, m, mp):
        w_hi = reindex(w_hi)
        out = np.zeros((128, KS, 3, 2, mp), E4M3)
        for pair in range(24):
            g, p0 = divmod(pair * 16, 128)
            out[p0:p0 + 16, :, g, 0, 0:m] = w_hi[pair]   # (16, KS, m)
            out[p0:p0 + 16, :, g, 1, 0:m] = w_hi[pair]
        return out

    def pack_lo(w_lo, m, mp):
        # term2 k-tile pairs: t=0 -> rows (0..127 | 128..255),
        # t=1 -> rows (256..383 | 384..399 zero-padded)
        w_lo = reindex(w_lo)
        out = np.zeros((128, KS, 2, 2, mp), E4M3)
        for pair in range(25):
            rho0 = pair * 16
            t, rem = divmod(rho0, 256)
            gg, p0 = divmod(rem, 128)
            out[p0:p0 + 16, :, t, gg, 0:m] = w_lo[pair]
        return out

    def pack_leftover_hi(w_hi, m, mp):
        # leftover pair2 = (i=4, k=4); rows (j, d)
        w_hi = reindex(w_hi)
        out = np.zeros((80, 2, mp), E4M3)
        for j in range(KS):
            out[16 * j:16 * j + 16, 0, 0:m] = w_hi[24, :, j, :]
            out[16 * j:16 * j + 16, 1, 0:m] = w_hi[24, :, j, :]
        return out

    wahi = pack_hi(WfA_hi, 120, 128)
    wbhi = pack_hi(kB_hi, 108, 112)
    walo = pack_lo(WfA_lo, 120, 128)
    wblo = pack_lo(kB_lo, 108, 112)
    wlahi = pack_leftover_hi(WfA_hi, 120, 128)
    wlbhi = pack_leftover_hi(kB_hi, 108, 112)

    # grid lhsT for mt0..2: rows (e*10+l, e<12); undo sA here
    gat = np.zeros((120, 3, 108), np.float32)
    for mt in range(3):
        for el2 in range(4):
            e = 4 * mt + el2
            for l in range(S):
                gat[e * S + l, mt, el2 * 27:(el2 + 1) * 27] = g2[:, l]
    gat = (gat / sA).astype(np.float16)

    # moment lhsTs: rows (el2*27+mln) -> col e = 4*mt+el2.
    # mt3 rows see sB-scaled relu values: undo with 1/sB^2 (num), 1/sB (den)
    wnd = np.zeros((108, 4, 16), np.float32)
    for mt in range(4):
        for el2 in range(4):
            e = 4 * mt + el2
            wnd[el2 * 27:(el2 + 1) * 27, mt, e] = w_flat
    wnd = wnd.astype(np.float16)

    bias_arr = np.asarray(bias, np.float32).reshape(16, 1)

    # input hi/lo quantization (global, then per-core packing)
    x32 = np.asarray(x, np.float32)
    x_hi8 = x32.astype(E4M3)
    x_lo8 = (x32 - x_hi8.astype(np.float32)).astype(E4M3)

    in_maps = []
    for c in range(NCORES):
        b, q = divmod(c, 4)
        sl = slice(q * SLAB, q * SLAB + SLAB_IN)
        shi = x_hi8[b, :, sl]                    # (16, 13, 40, 40) e4m3
        slo = x_lo8[b, :, sl]
        rhl = [np.zeros((128, SLAB, 2, D_IN, D_OUT), E4M3) for _ in range(3)]
        rh2 = [np.zeros((128, SLAB, 2, D_IN, D_OUT), E4M3) for _ in range(2)]
        for pair in range(25):
            i, k = divmod(pair, KS)
            hi_blk = shi[:, i:i + SLAB, :, k:k + D_OUT]      # (16,9,40,36)
            hi_blk = hi_blk.transpose(0, 1, 2, 3)
            if pair < 24:
                g, p0 = divmod(pair * 16, 128)
                lo_blk = slo[:, i:i + SLAB, :, k:k + D_OUT]
                rhl[g][p0:p0 + 16, :, 0] = hi_blk
                rhl[g][p0:p0 + 16, :, 1] = lo_blk
            t, rem = divmod(pair * 16, 256)
            gg, p0 = divmod(rem, 128)
            rh2[t][p0:p0 + 16, :, gg] = hi_blk
        rl = np.empty((80, SLAB, 2, D_OUT, D_OUT), E4M3)
        for j in range(KS):
            rl[16 * j:16 * j + 16, :, 0] = shi[:, 4:4 + SLAB, j:j + D_OUT,
                                               4:4 + D_OUT]
            rl[16 * j:16 * j + 16, :, 1] = slo[:, 4:4 + SLAB, j:j + D_OUT,
                                               4:4 + D_OUT]
        in_maps.append({
            "rhl0": rhl[0], "rhl1": rhl[1], "rhl2": rhl[2], "rl": rl,
            "rh20": rh2[0], "rh21": rh2[1],
            "wahi": wahi, "wbhi": wbhi, "wlahi": wlahi, "wlbhi": wlbhi,
            "walo": walo, "wblo": wblo,
            "gat": np.ascontiguousarray(gat),
            "wnd": np.ascontiguousarray(wnd),
            "bias": bias_arr,
        })
    return in_maps, sB


def _run(inputs, trace=False, **run_kwargs):
    inputs = {k: np.asarray(v) for k, v in inputs.items()}
    in_maps, sB = _host_prep(**inputs)
    if _prog_cache.get("sB") != float(sB):
        _prog_cache["nc"] = _build_program(1.0 / sB)
        _prog_cache["sB"] = float(sB)
    nc = _prog_cache["nc"]
    try:
        res = run_bass_kernel_spmd(nc, in_maps, core_ids=list(range(NCORES)),
                                   trace=trace, **run_kwargs)
    except ModuleNotFoundError as e:
        if "axon_hooks" not in str(e):
            raise
        os.environ["BASS_NEVER_TRACE"] = "1"
        res = run_bass_kernel_spmd(nc, in_maps, core_ids=list(range(NCORES)),
                                   trace=False, **run_kwargs)
    out = np.empty((B, COUT, D_OUT, D_OUT, D_OUT), np.float32)
    for c in range(NCORES):
        b, q = divmod(c, 4)
        out[b, :, q * SLAB:(q + 1) * SLAB] = res.results[c]["y"]
    return out, res


def kernel(**inputs):
    out, _ = _run(inputs)
    return out
